# revision 14
# baseline (speedup 1.0000x reference)
"""GAT-style 2-layer GNN message passing on 8 Trainium2 NeuronCores.

Math note: for this reference, the segment-softmax ratio
  num/den = (sum_j h[j]*exp((s_l[i]+s_r[j])/2d)) / (sum_j exp((s_l[i]+s_r[j])/2d))
has the destination factor exp(s_l[i]/2d) cancel, so per layer we only need
  a[i] = (sum_{j in N(i)} w_j*h_j) / (sum_{j in N(i)} w_j),  w_j = exp(s_r[j]/2d).

Sharding: nodes split into 8 contiguous destination ranges (6250/core).
Each core builds table rows [g=w*h (256) | w (4) | pad] (bf16, 768B) for its
own nodes; per layer the table is split in two halves by local row and
distributed with two pipelined AllGathers (layer 0 uses a small A half so
the first AllGather lands early; layer 1 uses a small B half so the last
AllGather before the final sweep is short). Each core aggregates its own
destinations: per-edge dma_gather of source rows (per-block chunk-exact
counts with -1 tail trim), then one-hot (fp8) matmul segment-sum into PSUM.
Early blocks (B-half gather still waiting on its AllGather) spill A-half
partials to an SBUF accumulator; steady-state blocks accumulate B directly
onto the A PSUM group. Layer-1 table rows are built inline in the layer-0
aggregation sweep via a PE transpose of x1.
"""

import os
import sys

import numpy as np
import ml_dtypes

sys.path.insert(0, "/opt/trn_rl_repo")

import concourse.bacc as bacc
import concourse.bass as bass
import concourse.mybir as mybir
import concourse.tile as tile
from concourse.bass_utils import run_bass_kernel_spmd

BF16 = mybir.dt.bfloat16
F32 = mybir.dt.float32
I16 = mybir.dt.int16
F8 = mybir.dt.float8e4 if os.environ.get("KF8", "1") == "1" else mybir.dt.bfloat16

N, DIN, E = 50000, 128, 800000
H, D = 4, 64
F = H * D  # 256
FH = F + H  # 260
NCORE = 8
NPC = N // NCORE  # 6250
NBLK = (NPC + 127) // 128  # 49 destination blocks per core
EPS = 1e-5
SLOPE = 0.01
ROWE = 384  # table row: 256 g + 4 w + 124 pad (bf16) = 768 bytes
# per-layer (A, B) split of the 6250 local rows; 8*size must fit int16
SPLITS = ((2176, 4074), (3968, 2282))
K1 = (1 + SLOPE) / 2 / (2 * D)
K2 = (1 - SLOPE) / 2 / (2 * D)

LAST_RESULTS = None

AF = mybir.ActivationFunctionType
ALU = mybir.AluOpType


def _host_prep(x, edge_index, W0, b0, W1, b1, att0, att1, gamma, beta):
    """Build all per-core and shared input arrays.

    Per (layer, core, block, half) the edge list indexes into that layer's
    half table. All cores share one static chunk count per slot (the max
    over cores); cores short of (nch-1)*128+1 edges are topped up with
    row-0 dummy edges (one-hot zero), then -1-padded to nch*128 so the
    gather ucode trims the tail consistently with the ring reservation.
    """
    bf16 = ml_dtypes.bfloat16
    f8 = (
        ml_dtypes.float8_e4m3fn
        if os.environ.get("KF8", "1") == "1"
        else ml_dtypes.bfloat16
    )
    dst = np.asarray(edge_index[0], dtype=np.int64)
    src = np.asarray(edge_index[1], dtype=np.int64)

    plain_ln = bool(
        np.allclose(np.asarray(gamma), 1.0) and np.allclose(np.asarray(beta), 0.0)
    )

    ksort = os.environ.get("KSORT", "0") == "1"
    # per_core[l][c][b][half] -> (rows, lb)
    per_core = [[], []]
    for l in range(2):
        sa, sb = SPLITS[l]
        for c in range(NCORE):
            m = (dst >= c * NPC) & (dst < (c + 1) * NPC)
            ld = dst[m] - c * NPC
            s = src[m]
            owner = s // NPC
            srow = s - owner * NPC
            inA = srow < sa
            idxA_all = owner * sa + srow
            idxB_all = owner * sb + (srow - sa)
            blocks = []
            for b in range(NBLK):
                bm = (ld >> 7) == b
                lb = ld[bm] & 127
                a_m = inA[bm]
                halves = []
                for rows_all, hm in ((idxA_all[bm], a_m), (idxB_all[bm], ~a_m)):
                    rows = rows_all[hm]
                    ll = lb[hm]
                    if ksort:
                        order = np.argsort(rows, kind="stable")
                        rows, ll = rows[order], ll[order]
                    halves.append((rows, ll))
                blocks.append(halves)
            per_core[l].append(blocks)

    # static chunk counts (max over cores), per layer/block/half
    nch = np.zeros((2, NBLK, 2), dtype=np.int64)
    for l in range(2):
        for b in range(NBLK):
            for r in range(2):
                mx = max(len(per_core[l][c][b][r][0]) for c in range(NCORE))
                nch[l, b, r] = max(1, -(-mx // 128))
    eoff = [
        [np.concatenate([[0], np.cumsum(nch[l, :, r] * 8)]) for r in range(2)]
        for l in range(2)
    ]
    ohoff = [
        [np.concatenate([[0], np.cumsum(nch[l, :, r] * 128)]) for r in range(2)]
        for l in range(2)
    ]

    eidx_all = [[[], []], [[], []]]  # [l][r][c]
    ohm_all = [[[], []], [[], []]]
    for l in range(2):
        for c in range(NCORE):
            for r in range(2):
                eidx = np.zeros((128, int(eoff[l][r][-1])), dtype=np.int16)
                ohm = np.zeros((128, int(ohoff[l][r][-1])), dtype=f8)
                for b in range(NBLK):
                    rows, lb = per_core[l][c][b][r]
                    n = len(rows)
                    cap = int(nch[l, b, r]) * 128
                    floor = (int(nch[l, b, r]) - 1) * 128 + 1
                    ntop = max(n, min(floor, cap))
                    si = np.full(cap, -1, dtype=np.int16)
                    si[:n] = rows.astype(np.int16)
                    si[n:ntop] = 0
                    e0, e1 = int(eoff[l][r][b]), int(eoff[l][r][b + 1])
                    eidx[:, e0:e1] = np.tile(si.reshape(-1, 16).T, (8, 1))
                    col0 = int(ohoff[l][r][b])
                    pos = np.arange(n)
                    ohm[pos % 128, col0 + (pos // 128) * 128 + lb.astype(np.int64)] = 1.0
                eidx_all[l][r].append(eidx)
                ohm_all[l][r].append(ohm)

    xts = []
    xf = np.asarray(x, dtype=np.float32)
    for c in range(NCORE):
        xt = np.zeros((DIN, NBLK * 128), dtype=bf16)
        xt[:, :NPC] = xf[c * NPC : (c + 1) * NPC].T.astype(bf16)
        xts.append(xt)

    shared = {
        "w0t": np.ascontiguousarray(np.asarray(W0, np.float32).T).astype(bf16),
        "w1t": np.ascontiguousarray(np.asarray(W1, np.float32).T).astype(bf16),
        "b0b": np.tile(np.asarray(b0, np.float32)[None, :], (128, 1)),
        "b1b": np.tile(np.asarray(b1, np.float32)[None, :], (128, 1)),
        "ar0": np.tile(
            np.asarray(att0, np.float32)[0, :, D:].reshape(-1)[None, :], (128, 1)
        ),
        "ar1": np.tile(
            np.asarray(att1, np.float32)[0, :, D:].reshape(-1)[None, :], (128, 1)
        ),
        "gmb": np.tile(np.asarray(gamma, np.float32)[None, :], (128, 1)),
        "btb": np.tile(np.asarray(beta, np.float32)[None, :], (128, 1)),
        "idn": np.eye(128, dtype=np.float32).astype(bf16),
    }
    in_maps = []
    for c in range(NCORE):
        m = dict(shared)
        m["xt"] = xts[c]
        for l in range(2):
            for r, hn in ((0, "A"), (1, "B")):
                m[f"eidx{hn}{l}"] = eidx_all[l][r][c]
                m[f"ohm{hn}{l}"] = ohm_all[l][r][c]
        in_maps.append(m)
    return in_maps, nch, plain_ln


def _build_program(nch, plain_ln):
    """nch: [2, NBLK, 2] static chunk counts per (layer, block, half)."""
    NCHA_MAX = int(nch[:, :, 0].max())
    NCHB_MAX = int(nch[:, :, 1].max())
    e16off = [
        [np.concatenate([[0], np.cumsum(nch[l, :, r] * 8)]) for r in range(2)]
        for l in range(2)
    ]
    ohoff = [
        [np.concatenate([[0], np.cumsum(nch[l, :, r] * 128)]) for r in range(2)]
        for l in range(2)
    ]
    NBLKA = [SPLITS[l][0] // 128 for l in range(2)]
    BST0 = int(os.environ.get("KB0", "22"))  # layer-0 first B-gather step
    BST1 = int(os.environ.get("KB1", "10"))  # layer-1 first B-gather step

    nc = bacc.Bacc(
        "TRN2",
        target_bir_lowering=False,
        debug=False,
        num_devices=NCORE,
        num_swdge_queues=4,
    )

    xt_d = nc.dram_tensor("xt", [DIN, NBLK * 128], BF16, kind="ExternalInput")
    eidx_d = [[None, None], [None, None]]
    ohm_d = [[None, None], [None, None]]
    for l in range(2):
        for r, hn in ((0, "A"), (1, "B")):
            eidx_d[l][r] = nc.dram_tensor(
                f"eidx{hn}{l}", [128, int(e16off[l][r][-1])], I16, kind="ExternalInput"
            )
            ohm_d[l][r] = nc.dram_tensor(
                f"ohm{hn}{l}", [128, int(ohoff[l][r][-1])], F8, kind="ExternalInput"
            )
    w0t_d = nc.dram_tensor("w0t", [DIN, F], BF16, kind="ExternalInput")
    w1t_d = nc.dram_tensor("w1t", [F, F], BF16, kind="ExternalInput")
    b0b_d = nc.dram_tensor("b0b", [128, F], F32, kind="ExternalInput")
    b1b_d = nc.dram_tensor("b1b", [128, F], F32, kind="ExternalInput")
    ar0_d = nc.dram_tensor("ar0", [128, F], F32, kind="ExternalInput")
    ar1_d = nc.dram_tensor("ar1", [128, F], F32, kind="ExternalInput")
    gmb_d = nc.dram_tensor("gmb", [128, F], F32, kind="ExternalInput")
    btb_d = nc.dram_tensor("btb", [128, F], F32, kind="ExternalInput")
    idn_d = nc.dram_tensor("idn", [128, 128], BF16, kind="ExternalInput")
    out_d = nc.dram_tensor("out", [NPC, D], F32, kind="ExternalOutput")

    tbl_own = [
        [
            nc.dram_tensor(f"tbl_own{l}{r}", [SPLITS[l][r], ROWE], BF16)
            for r in range(2)
        ]
        for l in range(2)
    ]
    tbl_full = [
        [
            nc.dram_tensor(
                f"tbl_full{l}{r}", [NCORE * SPLITS[l][r], ROWE], BF16,
                addr_space="Shared",
            )
            for r in range(2)
        ]
        for l in range(2)
    ]

    groups = [list(range(NCORE))]

    with tile.TileContext(nc) as tc:
        with (
            tc.tile_pool(name="const", bufs=1) as cpool,
            tc.tile_pool(name="hbuf", bufs=3) as hpool,
            tc.tile_pool(name="small", bufs=6) as smpool,
            tc.tile_pool(name="tblt", bufs=3) as tbpool,
            tc.tile_pool(name="ohp", bufs=3) as ohpool,
            tc.tile_pool(name="post", bufs=3) as postpool,
            tc.tile_pool(name="xtp", bufs=4) as xtpool,
            tc.tile_pool(name="gemm", bufs=1, space="PSUM") as gpsum,
            tc.tile_pool(name="agga", bufs=3, space="PSUM") as apsumA,
            tc.tile_pool(name="aggb", bufs=2, space="PSUM") as apsumB,
            tc.tile_pool(name="tp", bufs=2, space="PSUM") as tpsum,
        ):
            # ---- load constants ----
            def cload(dram, shape, dtype):
                t = cpool.tile(shape, dtype, tag=dram.name)
                nc.sync.dma_start(out=t[:], in_=dram[:, :])
                return t

            xt_s = cload(xt_d, [DIN, NBLK * 128], BF16)
            eidx_s = [
                [
                    cload(eidx_d[l][r], [128, int(e16off[l][r][-1])], I16)
                    for r in range(2)
                ]
                for l in range(2)
            ]
            w0t_s = cload(w0t_d, [DIN, F], BF16)
            w1t_s = []
            for cch in range(2):
                t = cpool.tile([128, F], BF16, tag=f"w1t{cch}")
                nc.sync.dma_start(out=t[:], in_=w1t_d[cch * 128 : (cch + 1) * 128, :])
                w1t_s.append(t)
            b0b_s = cload(b0b_d, [128, F], F32)
            b1b_s = cload(b1b_d, [128, F], F32)
            ar0_s = cload(ar0_d, [128, F], F32)
            ar1_s = cload(ar1_d, [128, F], F32)
            if not plain_ln:
                gmb_s = cload(gmb_d, [128, F], F32)
                btb_s = cload(btb_d, [128, F], F32)
            idn_s = cload(idn_d, [128, 128], BF16)
            epsb_s = cpool.tile([128, 1], F32, tag="epsb")
            nc.vector.memset(epsb_s[:], EPS)
            # A-half spill accumulator, one [128, FH] f32 slab per block
            acc_s = cpool.tile([128, NBLK, FH], F32, tag="acc")

            gtA = []
            gtB = []
            for i in range(3):
                t = cpool.tile([128, NCHA_MAX, ROWE], BF16, tag=f"gtA{i}")
                nc.vector.memset(t[:], 0.0)
                gtA.append(t)
                t = cpool.tile([128, NCHB_MAX, ROWE], BF16, tag=f"gtB{i}")
                nc.vector.memset(t[:], 0.0)
                gtB.append(t)

            regs = {}
            for v in sorted(set(nch.reshape(-1).tolist())):
                regs[int(v)] = nc.gpsimd.to_reg(int(v) * 128)

            def att_scores(u, dst_ap):
                """dst = su + (K2/K1)*sa, where su/sa are +/- abs row sums of u."""
                su = smpool.tile([128, H], F32, tag="su")
                nc.vector.tensor_reduce(
                    su[:],
                    u[:].rearrange("p (h d) -> p h d", d=D),
                    axis=mybir.AxisListType.X,
                    op=ALU.add,
                )
                sa = smpool.tile([128, H], F32, tag="sa")
                nc.vector.tensor_reduce(
                    sa[:],
                    u[:].rearrange("p (h d) -> p h d", d=D),
                    axis=mybir.AxisListType.X,
                    op=ALU.add,
                    apply_absolute_value=True,
                )
                t1 = smpool.tile([128, H], F32, tag="t1")
                nc.vector.tensor_scalar(t1[:], sa[:], K2 / K1, None, op0=ALU.mult)
                nc.vector.tensor_tensor(dst_ap, su[:], t1[:], op=ALU.add)

            def emit_rows(l, t, h, srf):
                """Build [g=w*h | w] row block and DMA it to the own table."""
                rows = min(128, NPC - t * 128)
                sa = SPLITS[l][0]
                tb = tbpool.tile([128, F + H], BF16, tag="tb")
                nc.scalar.activation(tb[:, F : F + H], srf, AF.Exp, scale=K1)
                nc.vector.tensor_tensor(
                    tb[:, 0:F].rearrange("p (h d) -> p h d", d=D),
                    h[:].rearrange("p (h d) -> p h d", d=D),
                    tb[:, F : F + H].to_broadcast((128, H, D)),
                    op=ALU.mult,
                )
                if t < NBLKA[l]:
                    dst = tbl_own[l][0][t * 128 : t * 128 + rows, 0 : F + H]
                else:
                    r0 = t * 128 - sa
                    dst = tbl_own[l][1][r0 : r0 + rows, 0 : F + H]
                nc.sync.dma_start(out=dst, in_=tb[:rows, :])

            def build0(t):
                """Layer-0 GEMM + table row for destination block t."""
                ps = gpsum.tile([128, F], F32, tag="gemm")
                nc.tensor.matmul(
                    ps[:],
                    lhsT=xt_s[:, t * 128 : (t + 1) * 128],
                    rhs=w0t_s[:],
                    start=True,
                    stop=True,
                )
                h = hpool.tile([128, F], F32, tag="h")
                nc.vector.tensor_tensor(h[:], ps[:], b0b_s[:], op=ALU.add)
                u = hpool.tile([128, F], F32, tag="u")
                nc.vector.tensor_tensor(u[:], h[:], ar0_s[:], op=ALU.mult)
                srf = smpool.tile([128, H], F32, tag="srf")
                att_scores(u, srf[:])
                emit_rows(0, t, h, srf[:])

            qctr = [0]

            def gather_half(l, t, r):
                """Issue one gather (r 0 = half A, 1 = half B) for block t."""
                qn = qctr[0] % 4
                qctr[0] += 1
                gt = (gtA if r == 0 else gtB)[t % 3]
                nchv = int(nch[l, t, r])
                e0 = int(e16off[l][r][t])
                nc.gpsimd.dma_gather(
                    gt[:, 0:nchv, :],
                    tbl_full[l][r][:, :],
                    eidx_s[l][r][:, e0 : e0 + nchv * 8],
                    nchv * 128,
                    regs[nchv],
                    ROWE,
                    single_packet=(nchv * 128 <= 1024),
                    queue_num=qn,
                )
                return gt

            def load_onehot(l, t, r):
                mx = NCHA_MAX if r == 0 else NCHB_MAX
                nchv = int(nch[l, t, r])
                oh = ohpool.tile([128, mx * 128], F8, tag=f"oh{r}")
                o0 = int(ohoff[l][r][t])
                nc.sync.dma_start(
                    out=oh[:, 0 : nchv * 128], in_=ohm_d[l][r][:, o0 : o0 + nchv * 128]
                )
                return oh

            def agg_matmuls(ps, gt, oh, l, t, r, start, stop):
                nchv = int(nch[l, t, r])
                for b in range(nchv):
                    nc.tensor.matmul(
                        ps[:],
                        lhsT=oh[:, b * 128 : (b + 1) * 128],
                        rhs=gt[:, b, 0:FH],
                        start=(start and b == 0),
                        stop=(stop and b == nchv - 1),
                    )

            ps_live = {}
            direct = {}

            def agg_front(l, t, is_direct):
                """A-half gather + matmuls; spill to SBUF unless direct."""
                ga = gather_half(l, t, 0)
                oh = load_onehot(l, t, 0)
                ps = apsumA.tile([128, FH], F32, tag="aggA")
                agg_matmuls(ps, ga, oh, l, t, 0, start=True, stop=not is_direct)
                direct[t] = is_direct
                if is_direct:
                    ps_live[t] = ps
                else:
                    nc.vector.tensor_copy(acc_s[:, t, :], ps[:])

            def agg_mid(l, t):
                gb = gather_half(l, t, 1)
                oh = load_onehot(l, t, 1)
                if direct[t]:
                    agg_matmuls(ps_live[t], gb, oh, l, t, 1, start=False, stop=True)
                else:
                    ps = apsumB.tile([128, FH], F32, tag="aggB")
                    agg_matmuls(ps, gb, oh, l, t, 1, start=True, stop=True)
                    ps_live[t] = ps

            def div_merge(t):
                """-> (num/den source tile, rec).  Merges the spill if needed."""
                ps = ps_live.pop(t)
                if direct.pop(t):
                    src = ps
                else:
                    s = postpool.tile([128, FH], F32, tag="s")
                    nc.vector.tensor_tensor(s[:], ps[:], acc_s[:, t, :], op=ALU.add)
                    src = s
                rec = smpool.tile([128, H], F32, tag="rec")
                nc.vector.reciprocal_approx_fast(out=rec[:], in_=src[:, F:FH])
                return src, rec

            x1T_live = {}

            def agg0_back(t):
                """num/den + LayerNorm + lrelu; PE-transpose x1 for the GEMM."""
                s, rec = div_merge(t)
                a0 = postpool.tile([128, F], F32, tag="a0")
                sm = smpool.tile([128, 1], F32, tag="sm")
                scr = postpool.tile([128, F], F32, tag="scr")
                sq = smpool.tile([128, 1], F32, tag="sq")
                if os.environ.get("KTTR", "0") == "1":
                    nc.vector.tensor_tensor_reduce(
                        a0[:].rearrange("p (h d) -> p h d", d=D),
                        s[:, 0:F].rearrange("p (h d) -> p h d", d=D),
                        rec[:].to_broadcast((128, H, D)),
                        1.0,
                        0.0,
                        op0=ALU.mult,
                        op1=ALU.add,
                        accum_out=sm[:],
                    )
                    nc.vector.tensor_tensor_reduce(
                        scr[:], a0[:], a0[:], 1.0, 0.0,
                        op0=ALU.mult, op1=ALU.add, accum_out=sq[:],
                    )
                else:
                    nc.vector.tensor_tensor(
                        a0[:].rearrange("p (h d) -> p h d", d=D),
                        s[:, 0:F].rearrange("p (h d) -> p h d", d=D),
                        rec[:].to_broadcast((128, H, D)),
                        op=ALU.mult,
                    )
                    nc.vector.tensor_reduce(
                        sm[:], a0[:], axis=mybir.AxisListType.X, op=ALU.add
                    )
                    nc.vector.tensor_tensor(scr[:], a0[:], a0[:], op=ALU.mult)
                    nc.vector.tensor_reduce(
                        sq[:], scr[:], axis=mybir.AxisListType.X, op=ALU.add
                    )
                mun = smpool.tile([128, 1], F32, tag="mun")
                nc.vector.tensor_scalar(mun[:], sm[:], -1.0 / F, None, op0=ALU.mult)
                m2 = smpool.tile([128, 1], F32, tag="m2")
                nc.vector.tensor_tensor(m2[:], mun[:], sm[:], op=ALU.mult)
                dv = smpool.tile([128, 1], F32, tag="dv")
                nc.vector.tensor_tensor(dv[:], sq[:], m2[:], op=ALU.add)
                rstd = smpool.tile([128, 1], F32, tag="rstd")
                nc.scalar.activation(
                    rstd[:], dv[:], AF.Abs_reciprocal_sqrt, bias=epsb_s[:], scale=1.0 / F
                )
                nmr = smpool.tile([128, 1], F32, tag="nmr")
                nc.vector.tensor_tensor(nmr[:], mun[:], rstd[:], op=ALU.mult)
                x1b = tbpool.tile([128, F], BF16, tag="x1b")
                if plain_ln:
                    nc.scalar.activation(
                        x1b[:], a0[:], AF.Lrelu, bias=nmr[:], scale=rstd[:], alpha=SLOPE
                    )
                else:
                    xn = postpool.tile([128, F], F32, tag="xn")
                    nc.scalar.activation(xn[:], a0[:], AF.Copy, bias=nmr[:], scale=rstd[:])
                    xg = postpool.tile([128, F], F32, tag="xg")
                    nc.vector.tensor_tensor(xg[:], xn[:], gmb_s[:], op=ALU.mult)
                    xgb = postpool.tile([128, F], F32, tag="xgb")
                    nc.vector.tensor_tensor(xgb[:], xg[:], btb_s[:], op=ALU.add)
                    nc.scalar.activation(x1b[:], xgb[:], AF.Lrelu, alpha=SLOPE)
                pt = tpsum.tile([128, F], BF16, tag="tp")
                for cch in range(2):
                    nc.tensor.transpose(
                        pt[:, cch * 128 : (cch + 1) * 128],
                        x1b[:, cch * 128 : (cch + 1) * 128],
                        idn_s[:],
                    )
                xtT = xtpool.tile([128, F], BF16, tag="xT")
                nc.vector.tensor_copy(xtT[:], pt[:])
                x1T_live[t] = xtT

            def agg0_back2(t):
                """Layer-1 GEMM from the transposed x1 + scores + table rows."""
                xtT = x1T_live.pop(t)
                ps2 = gpsum.tile([128, F], F32, tag="gemm")
                for cch in range(2):
                    nc.tensor.matmul(
                        ps2[:],
                        lhsT=xtT[:, cch * 128 : (cch + 1) * 128],
                        rhs=w1t_s[cch][:],
                        start=(cch == 0),
                        stop=(cch == 1),
                    )
                h1 = hpool.tile([128, F], F32, tag="h")
                nc.vector.tensor_tensor(h1[:], ps2[:], b1b_s[:], op=ALU.add)
                u1 = hpool.tile([128, F], F32, tag="u")
                nc.vector.tensor_tensor(u1[:], h1[:], ar1_s[:], op=ALU.mult)
                srf = smpool.tile([128, H], F32, tag="srf")
                att_scores(u1, srf[:])
                emit_rows(1, t, h1, srf[:])

            def agg1_back(t):
                """num/den then head mean for block t."""
                s, rec = div_merge(t)
                rows = min(128, NPC - t * 128)
                rec4 = smpool.tile([128, H], F32, tag="rec4")
                nc.vector.tensor_scalar(rec4[:], rec[:], 0.25, None, op0=ALU.mult)
                q = postpool.tile([128, F], F32, tag="a0")
                nc.vector.tensor_tensor(
                    q[:].rearrange("p (h d) -> p h d", d=D),
                    s[:, 0:F].rearrange("p (h d) -> p h d", d=D),
                    rec4[:].to_broadcast((128, H, D)),
                    op=ALU.mult,
                )
                p01 = postpool.tile([128, D], F32, tag="p01")
                nc.vector.tensor_tensor(p01[:], q[:, 0:D], q[:, D : 2 * D], op=ALU.add)
                p23 = postpool.tile([128, D], F32, tag="p23")
                nc.vector.tensor_tensor(
                    p23[:], q[:, 2 * D : 3 * D], q[:, 3 * D : 4 * D], op=ALU.add
                )
                o = postpool.tile([128, D], F32, tag="o")
                nc.vector.tensor_tensor(o[:], p01[:], p23[:], op=ALU.add)
                nc.sync.dma_start(
                    out=out_d[t * 128 : t * 128 + rows, :], in_=o[:rows, :]
                )

            def allgather(l, half):
                nc.gpsimd.collective_compute(
                    "AllGather",
                    ALU.bypass,
                    replica_groups=groups,
                    ins=[tbl_own[l][half][:, :]],
                    outs=[tbl_full[l][half][:, :]],
                )

            # ================= schedule =================
            def mid_steps(bstart):
                """mid step per block: catch-up (2 mids/step) from bstart."""
                ms = {}
                avail = bstart
                for t in range(NBLK):
                    s = max(t + 2, avail)
                    ms.setdefault(s, []).append(t)
                    avail = s + 1 if len(ms[s]) >= 2 else s
                return ms

            def agg_sweep(l, back, back2, bstart, post_a=None, post_all=None):
                ms = mid_steps(bstart)
                mstep = {t: s for s, ts in ms.items() for t in ts}
                last_step = max(ms) + (2 if back2 else 0)
                for step in range(0, last_step + 1):
                    for t in ms.get(step, []):
                        agg_mid(l, t)
                    # back2 first: its inputs are 2 steps old, so it gives the
                    # DVE/PE queues work while back() waits on the B PSUM stop.
                    if back2 is not None:
                        for t in ms.get(step - 2, []):
                            back2(t)
                            if post_a is not None and t == NBLKA[1] - 1:
                                post_a()
                            if post_all is not None and t == NBLK - 1:
                                post_all()
                    for t in ms.get(step, []):
                        back(t)
                    if step < NBLK:
                        # direct accumulation once mids have caught up close
                        agg_front(
                            l,
                            step,
                            os.environ.get("KHYB", "1") == "1"
                            and mstep[step] - step <= 3,
                        )

            # ---- layer 0 build ----
            for t in range(NBLKA[0]):
                build0(t)
            allgather(0, 0)
            for t in range(NBLKA[0], NBLK):
                build0(t)
            allgather(0, 1)
            # ---- layer 0 aggregate + layer 1 build (fused) ----
            agg_sweep(
                0,
                agg0_back,
                agg0_back2,
                BST0,
                post_a=lambda: allgather(1, 0),
                post_all=lambda: allgather(1, 1),
            )
            # ---- layer 1 aggregate ----
            agg_sweep(1, agg1_back, None, BST1)

    nc.compile()
    return nc


_CACHE = {}


def kernel(**inputs):
    global LAST_RESULTS
    in_maps, nch, plain_ln = _host_prep(**inputs)
    key = (
        tuple(nch.reshape(-1).tolist()),
        plain_ln,
        os.environ.get("KB0"),
        os.environ.get("KB1"),
        os.environ.get("KSORT"),
        os.environ.get("KF8"),
        os.environ.get("KHYB"),
        os.environ.get("KTTR"),
    )
    if key not in _CACHE:
        _CACHE[key] = _build_program(nch, plain_ln)
    nc = _CACHE[key]
    trace = bool(os.environ.get("BASS_TRACE"))
    res = run_bass_kernel_spmd(nc, in_maps, list(range(NCORE)), trace=trace)
    LAST_RESULTS = res
    out = np.concatenate([res.results[c]["out"] for c in range(NCORE)], axis=0)
    return out.astype(np.float32)


# revision 16
# speedup vs baseline: 1.0133x; 1.0133x over previous
"""GAT-style 2-layer GNN message passing on 8 Trainium2 NeuronCores.

Math note: for this reference, the segment-softmax ratio
  num/den = (sum_j h[j]*exp((s_l[i]+s_r[j])/2d)) / (sum_j exp((s_l[i]+s_r[j])/2d))
has the destination factor exp(s_l[i]/2d) cancel, so per layer we only need
  a[i] = (sum_{j in N(i)} w_j*h_j) / (sum_{j in N(i)} w_j),  w_j = exp(s_r[j]/2d).

Sharding: nodes split into 8 contiguous destination ranges (6250/core).
Each core builds table rows [g=w*h (256) | w (4) | pad] (bf16, 768B) for its
own nodes; per layer the table is split in two halves by local row and
distributed with two pipelined AllGathers (layer 0 uses a small A half so
the first AllGather lands early; layer 1 uses a small B half so the last
AllGather before the final sweep is short). Each core aggregates its own
destinations: per-edge dma_gather of source rows (per-block chunk-exact
counts with -1 tail trim), then one-hot (fp8) matmul segment-sum into PSUM.
Early blocks (B-half gather still waiting on its AllGather) spill A-half
partials to an SBUF accumulator; steady-state blocks accumulate B directly
onto the A PSUM group. Layer-1 table rows are built inline in the layer-0
aggregation sweep via a PE transpose of x1.
"""

import os
import sys

import numpy as np
import ml_dtypes

sys.path.insert(0, "/opt/trn_rl_repo")

import concourse.bacc as bacc
import concourse.bass as bass
import concourse.mybir as mybir
import concourse.tile as tile
from concourse.bass_utils import run_bass_kernel_spmd

BF16 = mybir.dt.bfloat16
F32 = mybir.dt.float32
I16 = mybir.dt.int16
F8 = mybir.dt.float8e4 if os.environ.get("KF8", "1") == "1" else mybir.dt.bfloat16
F8R = mybir.dt.float8e4

N, DIN, E = 50000, 128, 800000
H, D = 4, 64
F = H * D  # 256
FH = F + H  # 260
NCORE = 8
NPC = N // NCORE  # 6250
NBLK = (NPC + 127) // 128  # 49 destination blocks per core
EPS = 1e-5
SLOPE = 0.01
ROWE = 512  # table row: 256 g + 4 (w-1) fp8 + 252 pad = 512 bytes
# per-layer (A, B) split of the 6250 local rows; 8*size must fit int16
SPLITS = ((2176, 4074), (3968, 2282))
K1 = (1 + SLOPE) / 2 / (2 * D)
K2 = (1 - SLOPE) / 2 / (2 * D)

LAST_RESULTS = None

AF = mybir.ActivationFunctionType
ALU = mybir.AluOpType


def _host_prep(x, edge_index, W0, b0, W1, b1, att0, att1, gamma, beta):
    """Build all per-core and shared input arrays.

    Per (layer, core, block, half) the edge list indexes into that layer's
    half table. All cores share one static chunk count per slot (the max
    over cores); cores short of (nch-1)*128+1 edges are topped up with
    row-0 dummy edges (one-hot zero), then -1-padded to nch*128 so the
    gather ucode trims the tail consistently with the ring reservation.
    """
    bf16 = ml_dtypes.bfloat16
    f8 = (
        ml_dtypes.float8_e4m3fn
        if os.environ.get("KF8", "1") == "1"
        else ml_dtypes.bfloat16
    )
    dst = np.asarray(edge_index[0], dtype=np.int64)
    src = np.asarray(edge_index[1], dtype=np.int64)

    plain_ln = bool(
        np.allclose(np.asarray(gamma), 1.0) and np.allclose(np.asarray(beta), 0.0)
    )

    ksort = os.environ.get("KSORT", "0") == "1"
    # per_core[l][c][b][half] -> (rows, lb)
    per_core = [[], []]
    for l in range(2):
        sa, sb = SPLITS[l]
        for c in range(NCORE):
            m = (dst >= c * NPC) & (dst < (c + 1) * NPC)
            ld = dst[m] - c * NPC
            s = src[m]
            owner = s // NPC
            srow = s - owner * NPC
            inA = srow < sa
            idxA_all = owner * sa + srow
            idxB_all = owner * sb + (srow - sa)
            blocks = []
            for b in range(NBLK):
                bm = (ld >> 7) == b
                lb = ld[bm] & 127
                a_m = inA[bm]
                halves = []
                for rows_all, hm in ((idxA_all[bm], a_m), (idxB_all[bm], ~a_m)):
                    rows = rows_all[hm]
                    ll = lb[hm]
                    if ksort:
                        order = np.argsort(rows, kind="stable")
                        rows, ll = rows[order], ll[order]
                    halves.append((rows, ll))
                blocks.append(halves)
            per_core[l].append(blocks)

    # static chunk counts (max over cores), per layer/block/half
    nch = np.zeros((2, NBLK, 2), dtype=np.int64)
    for l in range(2):
        for b in range(NBLK):
            for r in range(2):
                mx = max(len(per_core[l][c][b][r][0]) for c in range(NCORE))
                nch[l, b, r] = max(1, -(-mx // 128))
    eoff = [
        [np.concatenate([[0], np.cumsum(nch[l, :, r] * 8)]) for r in range(2)]
        for l in range(2)
    ]
    ohoff = [
        [np.concatenate([[0], np.cumsum(nch[l, :, r] * 128)]) for r in range(2)]
        for l in range(2)
    ]

    eidx_all = [[[], []], [[], []]]  # [l][r][c]
    ohm_all = [[[], []], [[], []]]
    for l in range(2):
        for c in range(NCORE):
            for r in range(2):
                eidx = np.zeros((128, int(eoff[l][r][-1])), dtype=np.int16)
                ohm = np.zeros((128, int(ohoff[l][r][-1])), dtype=f8)
                for b in range(NBLK):
                    rows, lb = per_core[l][c][b][r]
                    n = len(rows)
                    cap = int(nch[l, b, r]) * 128
                    floor = (int(nch[l, b, r]) - 1) * 128 + 1
                    ntop = max(n, min(floor, cap))
                    si = np.full(cap, -1, dtype=np.int16)
                    si[:n] = rows.astype(np.int16)
                    si[n:ntop] = 0
                    e0, e1 = int(eoff[l][r][b]), int(eoff[l][r][b + 1])
                    eidx[:, e0:e1] = np.tile(si.reshape(-1, 16).T, (8, 1))
                    col0 = int(ohoff[l][r][b])
                    pos = np.arange(n)
                    ohm[pos % 128, col0 + (pos // 128) * 128 + lb.astype(np.int64)] = 1.0
                eidx_all[l][r].append(eidx)
                ohm_all[l][r].append(ohm)

    cnt = np.zeros(N, np.float32)
    np.add.at(cnt, dst, 1.0)
    cnts = []
    for c in range(NCORE):
        pad = np.zeros(NBLK * 128, np.float32)
        pad[:NPC] = cnt[c * NPC : (c + 1) * NPC]
        cnts.append(np.ascontiguousarray(pad.reshape(NBLK, 128).T))
    xts = []
    xf = np.asarray(x, dtype=np.float32)
    for c in range(NCORE):
        xt = np.zeros((DIN, NBLK * 128), dtype=bf16)
        xt[:, :NPC] = xf[c * NPC : (c + 1) * NPC].T.astype(bf16)
        xts.append(xt)

    shared = {
        "w0t": np.ascontiguousarray(np.asarray(W0, np.float32).T).astype(bf16),
        "w1t": np.ascontiguousarray(np.asarray(W1, np.float32).T).astype(bf16),
        "b0b": np.tile(np.asarray(b0, np.float32)[None, :], (128, 1)),
        "b1b": np.tile(np.asarray(b1, np.float32)[None, :], (128, 1)),
        "ar0": np.tile(
            np.asarray(att0, np.float32)[0, :, D:].reshape(-1)[None, :], (128, 1)
        ),
        "ar1": np.tile(
            np.asarray(att1, np.float32)[0, :, D:].reshape(-1)[None, :], (128, 1)
        ),
        "gmb": np.tile(np.asarray(gamma, np.float32)[None, :], (128, 1)),
        "btb": np.tile(np.asarray(beta, np.float32)[None, :], (128, 1)),
        "idn": np.eye(128, dtype=np.float32).astype(bf16),
    }
    in_maps = []
    for c in range(NCORE):
        m = dict(shared)
        m["xt"] = xts[c]
        m["cnt"] = cnts[c]
        for l in range(2):
            for r, hn in ((0, "A"), (1, "B")):
                m[f"eidx{hn}{l}"] = eidx_all[l][r][c]
                m[f"ohm{hn}{l}"] = ohm_all[l][r][c]
        in_maps.append(m)
    return in_maps, nch, plain_ln


def _build_program(nch, plain_ln):
    """nch: [2, NBLK, 2] static chunk counts per (layer, block, half)."""
    NCHA_MAX = int(nch[:, :, 0].max())
    NCHB_MAX = int(nch[:, :, 1].max())
    e16off = [
        [np.concatenate([[0], np.cumsum(nch[l, :, r] * 8)]) for r in range(2)]
        for l in range(2)
    ]
    ohoff = [
        [np.concatenate([[0], np.cumsum(nch[l, :, r] * 128)]) for r in range(2)]
        for l in range(2)
    ]
    NBLKA = [SPLITS[l][0] // 128 for l in range(2)]
    BST0 = int(os.environ.get("KB0", "22"))  # layer-0 first B-gather step
    BST1 = int(os.environ.get("KB1", "10"))  # layer-1 first B-gather step

    nc = bacc.Bacc(
        "TRN2",
        target_bir_lowering=False,
        debug=False,
        num_devices=NCORE,
        num_swdge_queues=4,
    )

    xt_d = nc.dram_tensor("xt", [DIN, NBLK * 128], BF16, kind="ExternalInput")
    eidx_d = [[None, None], [None, None]]
    ohm_d = [[None, None], [None, None]]
    for l in range(2):
        for r, hn in ((0, "A"), (1, "B")):
            eidx_d[l][r] = nc.dram_tensor(
                f"eidx{hn}{l}", [128, int(e16off[l][r][-1])], I16, kind="ExternalInput"
            )
            ohm_d[l][r] = nc.dram_tensor(
                f"ohm{hn}{l}", [128, int(ohoff[l][r][-1])], F8, kind="ExternalInput"
            )
    w0t_d = nc.dram_tensor("w0t", [DIN, F], BF16, kind="ExternalInput")
    w1t_d = nc.dram_tensor("w1t", [F, F], BF16, kind="ExternalInput")
    b0b_d = nc.dram_tensor("b0b", [128, F], F32, kind="ExternalInput")
    b1b_d = nc.dram_tensor("b1b", [128, F], F32, kind="ExternalInput")
    ar0_d = nc.dram_tensor("ar0", [128, F], F32, kind="ExternalInput")
    ar1_d = nc.dram_tensor("ar1", [128, F], F32, kind="ExternalInput")
    gmb_d = nc.dram_tensor("gmb", [128, F], F32, kind="ExternalInput")
    btb_d = nc.dram_tensor("btb", [128, F], F32, kind="ExternalInput")
    idn_d = nc.dram_tensor("idn", [128, 128], BF16, kind="ExternalInput")
    cnt_d = nc.dram_tensor("cnt", [128, NBLK], F32, kind="ExternalInput")
    out_d = nc.dram_tensor("out", [NPC, D], F32, kind="ExternalOutput")

    tbl_own = [
        [
            nc.dram_tensor(f"tbl_own{l}{r}", [SPLITS[l][r], ROWE], F8R)
            for r in range(2)
        ]
        for l in range(2)
    ]
    tbl_full = [
        [
            nc.dram_tensor(
                f"tbl_full{l}{r}", [NCORE * SPLITS[l][r], ROWE], F8R,
                addr_space="Shared",
            )
            for r in range(2)
        ]
        for l in range(2)
    ]

    groups = [list(range(NCORE))]

    with tile.TileContext(nc) as tc:
        with (
            tc.tile_pool(name="const", bufs=1) as cpool,
            tc.tile_pool(name="hbuf", bufs=3) as hpool,
            tc.tile_pool(name="small", bufs=6) as smpool,
            tc.tile_pool(name="tblt", bufs=3) as tbpool,
            tc.tile_pool(name="ohp", bufs=3) as ohpool,
            tc.tile_pool(name="post", bufs=3) as postpool,
            tc.tile_pool(name="xtp", bufs=4) as xtpool,
            tc.tile_pool(name="gemm", bufs=1, space="PSUM") as gpsum,
            tc.tile_pool(name="agga", bufs=3, space="PSUM") as apsumA,
            tc.tile_pool(name="aggb", bufs=2, space="PSUM") as apsumB,
            tc.tile_pool(name="tp", bufs=2, space="PSUM") as tpsum,
        ):
            # ---- load constants ----
            def cload(dram, shape, dtype):
                t = cpool.tile(shape, dtype, tag=dram.name)
                nc.sync.dma_start(out=t[:], in_=dram[:, :])
                return t

            xt_s = cload(xt_d, [DIN, NBLK * 128], BF16)
            eidx_s = [
                [
                    cload(eidx_d[l][r], [128, int(e16off[l][r][-1])], I16)
                    for r in range(2)
                ]
                for l in range(2)
            ]
            w0t_s = cload(w0t_d, [DIN, F], BF16)
            w1t_s = []
            for cch in range(2):
                t = cpool.tile([128, F], BF16, tag=f"w1t{cch}")
                nc.sync.dma_start(out=t[:], in_=w1t_d[cch * 128 : (cch + 1) * 128, :])
                w1t_s.append(t)
            b0b_s = cload(b0b_d, [128, F], F32)
            b1b_s = cload(b1b_d, [128, F], F32)
            ar0_s = cload(ar0_d, [128, F], F32)
            ar1_s = cload(ar1_d, [128, F], F32)
            if not plain_ln:
                gmb_s = cload(gmb_d, [128, F], F32)
                btb_s = cload(btb_d, [128, F], F32)
            idn_s = cload(idn_d, [128, 128], BF16)
            cnt_s = cload(cnt_d, [128, NBLK], F32)
            epsb_s = cpool.tile([128, 1], F32, tag="epsb")
            nc.vector.memset(epsb_s[:], EPS)
            # A-half spill accumulator, one [128, FH] f32 slab per block
            acc_s = cpool.tile([128, NBLK, FH], F32, tag="acc")

            gtA = []
            gtB = []
            for i in range(3):
                t = cpool.tile([128, NCHA_MAX, ROWE], F8R, tag=f"gtA{i}")
                nc.vector.memset(t[:], 0.0)
                gtA.append(t)
                t = cpool.tile([128, NCHB_MAX, ROWE], F8R, tag=f"gtB{i}")
                nc.vector.memset(t[:], 0.0)
                gtB.append(t)

            regs = {}
            for v in sorted(set(nch.reshape(-1).tolist())):
                regs[int(v)] = nc.gpsimd.to_reg(int(v) * 128)

            def att_scores(u, dst_ap):
                """dst = su + (K2/K1)*sa, where su/sa are +/- abs row sums of u."""
                su = smpool.tile([128, H], F32, tag="su")
                nc.vector.tensor_reduce(
                    su[:],
                    u[:].rearrange("p (h d) -> p h d", d=D),
                    axis=mybir.AxisListType.X,
                    op=ALU.add,
                )
                sa = smpool.tile([128, H], F32, tag="sa")
                nc.vector.tensor_reduce(
                    sa[:],
                    u[:].rearrange("p (h d) -> p h d", d=D),
                    axis=mybir.AxisListType.X,
                    op=ALU.add,
                    apply_absolute_value=True,
                )
                t1 = smpool.tile([128, H], F32, tag="t1")
                nc.vector.tensor_scalar(t1[:], sa[:], K2 / K1, None, op0=ALU.mult)
                nc.vector.tensor_tensor(dst_ap, su[:], t1[:], op=ALU.add)

            def emit_rows(l, t, h, srf):
                """Build [g=w*h | w] row block and DMA it to the own table."""
                rows = min(128, NPC - t * 128)
                sa = SPLITS[l][0]
                tb = tbpool.tile([128, F + H], F8R, tag="tb")
                wv = smpool.tile([128, H], F32, tag="wv")
                nc.scalar.activation(wv[:], srf, AF.Exp, scale=K1)
                nc.vector.tensor_scalar(
                    tb[:, F : F + H], wv[:], -1.0, None, op0=ALU.add
                )
                nc.vector.tensor_tensor(
                    tb[:, 0:F].rearrange("p (h d) -> p h d", d=D),
                    h[:].rearrange("p (h d) -> p h d", d=D),
                    wv[:].to_broadcast((128, H, D)),
                    op=ALU.mult,
                )
                if t < NBLKA[l]:
                    dst = tbl_own[l][0][t * 128 : t * 128 + rows, 0 : F + H]
                else:
                    r0 = t * 128 - sa
                    dst = tbl_own[l][1][r0 : r0 + rows, 0 : F + H]
                nc.sync.dma_start(out=dst, in_=tb[:rows, :])

            def build0(t):
                """Layer-0 GEMM + table row for destination block t."""
                ps = gpsum.tile([128, F], F32, tag="gemm")
                nc.tensor.matmul(
                    ps[:],
                    lhsT=xt_s[:, t * 128 : (t + 1) * 128],
                    rhs=w0t_s[:],
                    start=True,
                    stop=True,
                )
                h = hpool.tile([128, F], F32, tag="h")
                nc.vector.tensor_tensor(h[:], ps[:], b0b_s[:], op=ALU.add)
                u = hpool.tile([128, F], F32, tag="u")
                nc.vector.tensor_tensor(u[:], h[:], ar0_s[:], op=ALU.mult)
                srf = smpool.tile([128, H], F32, tag="srf")
                att_scores(u, srf[:])
                emit_rows(0, t, h, srf[:])

            qctr = [0]

            def gather_half(l, t, r):
                """Issue one gather (r 0 = half A, 1 = half B) for block t."""
                qn = qctr[0] % 4
                qctr[0] += 1
                gt = (gtA if r == 0 else gtB)[t % 3]
                nchv = int(nch[l, t, r])
                e0 = int(e16off[l][r][t])
                nc.gpsimd.dma_gather(
                    gt[:, 0:nchv, :],
                    tbl_full[l][r][:, :],
                    eidx_s[l][r][:, e0 : e0 + nchv * 8],
                    nchv * 128,
                    regs[nchv],
                    ROWE,
                    single_packet=(nchv * 128 <= 1024),
                    queue_num=qn,
                )
                return gt

            def load_onehot(l, t, r):
                mx = NCHA_MAX if r == 0 else NCHB_MAX
                nchv = int(nch[l, t, r])
                oh = ohpool.tile([128, mx * 128], F8, tag=f"oh{r}")
                o0 = int(ohoff[l][r][t])
                nc.sync.dma_start(
                    out=oh[:, 0 : nchv * 128], in_=ohm_d[l][r][:, o0 : o0 + nchv * 128]
                )
                return oh

            def agg_matmuls(ps, gt, oh, l, t, r, start, stop):
                nchv = int(nch[l, t, r])
                for b in range(nchv):
                    nc.tensor.matmul(
                        ps[:],
                        lhsT=oh[:, b * 128 : (b + 1) * 128],
                        rhs=gt[:, b, 0:FH],
                        start=(start and b == 0),
                        stop=(stop and b == nchv - 1),
                    )

            ps_live = {}
            direct = {}

            def agg_front(l, t, is_direct):
                """A-half gather + matmuls; spill to SBUF unless direct."""
                ga = gather_half(l, t, 0)
                oh = load_onehot(l, t, 0)
                ps = apsumA.tile([128, FH], F32, tag="aggA")
                agg_matmuls(ps, ga, oh, l, t, 0, start=True, stop=not is_direct)
                direct[t] = is_direct
                if is_direct:
                    ps_live[t] = ps
                else:
                    nc.vector.tensor_copy(acc_s[:, t, :], ps[:])

            def agg_mid(l, t):
                gb = gather_half(l, t, 1)
                oh = load_onehot(l, t, 1)
                if direct[t]:
                    agg_matmuls(ps_live[t], gb, oh, l, t, 1, start=False, stop=True)
                else:
                    ps = apsumB.tile([128, FH], F32, tag="aggB")
                    agg_matmuls(ps, gb, oh, l, t, 1, start=True, stop=True)
                    ps_live[t] = ps

            def div_merge(t):
                """-> (num/den source tile, rec).  Merges the spill if needed."""
                ps = ps_live.pop(t)
                if direct.pop(t):
                    src = ps
                else:
                    s = postpool.tile([128, FH], F32, tag="s")
                    nc.vector.tensor_tensor(s[:], ps[:], acc_s[:, t, :], op=ALU.add)
                    src = s
                dn = smpool.tile([128, H], F32, tag="dn")
                nc.vector.tensor_tensor(
                    dn[:], src[:, F:FH], cnt_s[:, t : t + 1].to_broadcast((128, H)),
                    op=ALU.add,
                )
                rec = smpool.tile([128, H], F32, tag="rec")
                nc.vector.reciprocal_approx_fast(out=rec[:], in_=dn[:])
                return src, rec

            x1T_live = {}

            def agg0_back(t):
                """num/den + LayerNorm + lrelu; PE-transpose x1 for the GEMM."""
                s, rec = div_merge(t)
                a0 = postpool.tile([128, F], F32, tag="a0")
                sm = smpool.tile([128, 1], F32, tag="sm")
                scr = postpool.tile([128, F], F32, tag="scr")
                sq = smpool.tile([128, 1], F32, tag="sq")
                if os.environ.get("KTTR", "0") == "1":
                    nc.vector.tensor_tensor_reduce(
                        a0[:].rearrange("p (h d) -> p h d", d=D),
                        s[:, 0:F].rearrange("p (h d) -> p h d", d=D),
                        rec[:].to_broadcast((128, H, D)),
                        1.0,
                        0.0,
                        op0=ALU.mult,
                        op1=ALU.add,
                        accum_out=sm[:],
                    )
                    nc.vector.tensor_tensor_reduce(
                        scr[:], a0[:], a0[:], 1.0, 0.0,
                        op0=ALU.mult, op1=ALU.add, accum_out=sq[:],
                    )
                else:
                    nc.vector.tensor_tensor(
                        a0[:].rearrange("p (h d) -> p h d", d=D),
                        s[:, 0:F].rearrange("p (h d) -> p h d", d=D),
                        rec[:].to_broadcast((128, H, D)),
                        op=ALU.mult,
                    )
                    nc.vector.tensor_reduce(
                        sm[:], a0[:], axis=mybir.AxisListType.X, op=ALU.add
                    )
                    nc.vector.tensor_tensor(scr[:], a0[:], a0[:], op=ALU.mult)
                    nc.vector.tensor_reduce(
                        sq[:], scr[:], axis=mybir.AxisListType.X, op=ALU.add
                    )
                mun = smpool.tile([128, 1], F32, tag="mun")
                nc.vector.tensor_scalar(mun[:], sm[:], -1.0 / F, None, op0=ALU.mult)
                m2 = smpool.tile([128, 1], F32, tag="m2")
                nc.vector.tensor_tensor(m2[:], mun[:], sm[:], op=ALU.mult)
                dv = smpool.tile([128, 1], F32, tag="dv")
                nc.vector.tensor_tensor(dv[:], sq[:], m2[:], op=ALU.add)
                rstd = smpool.tile([128, 1], F32, tag="rstd")
                nc.scalar.activation(
                    rstd[:], dv[:], AF.Abs_reciprocal_sqrt, bias=epsb_s[:], scale=1.0 / F
                )
                nmr = smpool.tile([128, 1], F32, tag="nmr")
                nc.vector.tensor_tensor(nmr[:], mun[:], rstd[:], op=ALU.mult)
                x1b = tbpool.tile([128, F], BF16, tag="x1b")
                if plain_ln:
                    nc.scalar.activation(
                        x1b[:], a0[:], AF.Lrelu, bias=nmr[:], scale=rstd[:], alpha=SLOPE
                    )
                else:
                    xn = postpool.tile([128, F], F32, tag="xn")
                    nc.scalar.activation(xn[:], a0[:], AF.Copy, bias=nmr[:], scale=rstd[:])
                    xg = postpool.tile([128, F], F32, tag="xg")
                    nc.vector.tensor_tensor(xg[:], xn[:], gmb_s[:], op=ALU.mult)
                    xgb = postpool.tile([128, F], F32, tag="xgb")
                    nc.vector.tensor_tensor(xgb[:], xg[:], btb_s[:], op=ALU.add)
                    nc.scalar.activation(x1b[:], xgb[:], AF.Lrelu, alpha=SLOPE)
                pt = tpsum.tile([128, F], BF16, tag="tp")
                for cch in range(2):
                    nc.tensor.transpose(
                        pt[:, cch * 128 : (cch + 1) * 128],
                        x1b[:, cch * 128 : (cch + 1) * 128],
                        idn_s[:],
                    )
                xtT = xtpool.tile([128, F], BF16, tag="xT")
                nc.vector.tensor_copy(xtT[:], pt[:])
                x1T_live[t] = xtT

            def agg0_back2(t):
                """Layer-1 GEMM from the transposed x1 + scores + table rows."""
                xtT = x1T_live.pop(t)
                ps2 = gpsum.tile([128, F], F32, tag="gemm")
                for cch in range(2):
                    nc.tensor.matmul(
                        ps2[:],
                        lhsT=xtT[:, cch * 128 : (cch + 1) * 128],
                        rhs=w1t_s[cch][:],
                        start=(cch == 0),
                        stop=(cch == 1),
                    )
                h1 = hpool.tile([128, F], F32, tag="h")
                nc.vector.tensor_tensor(h1[:], ps2[:], b1b_s[:], op=ALU.add)
                u1 = hpool.tile([128, F], F32, tag="u")
                nc.vector.tensor_tensor(u1[:], h1[:], ar1_s[:], op=ALU.mult)
                srf = smpool.tile([128, H], F32, tag="srf")
                att_scores(u1, srf[:])
                emit_rows(1, t, h1, srf[:])

            def agg1_back(t):
                """num/den then head mean for block t."""
                s, rec = div_merge(t)
                rows = min(128, NPC - t * 128)
                rec4 = smpool.tile([128, H], F32, tag="rec4")
                nc.vector.tensor_scalar(rec4[:], rec[:], 0.25, None, op0=ALU.mult)
                q = postpool.tile([128, F], F32, tag="a0")
                nc.vector.tensor_tensor(
                    q[:].rearrange("p (h d) -> p h d", d=D),
                    s[:, 0:F].rearrange("p (h d) -> p h d", d=D),
                    rec4[:].to_broadcast((128, H, D)),
                    op=ALU.mult,
                )
                p01 = postpool.tile([128, D], F32, tag="p01")
                nc.vector.tensor_tensor(p01[:], q[:, 0:D], q[:, D : 2 * D], op=ALU.add)
                p23 = postpool.tile([128, D], F32, tag="p23")
                nc.vector.tensor_tensor(
                    p23[:], q[:, 2 * D : 3 * D], q[:, 3 * D : 4 * D], op=ALU.add
                )
                o = postpool.tile([128, D], F32, tag="o")
                nc.vector.tensor_tensor(o[:], p01[:], p23[:], op=ALU.add)
                nc.sync.dma_start(
                    out=out_d[t * 128 : t * 128 + rows, :], in_=o[:rows, :]
                )

            def allgather(l, half):
                nc.gpsimd.collective_compute(
                    "AllGather",
                    ALU.bypass,
                    replica_groups=groups,
                    ins=[tbl_own[l][half][:, :]],
                    outs=[tbl_full[l][half][:, :]],
                )

            # ================= schedule =================
            def mid_steps(bstart):
                """mid step per block: catch-up (2 mids/step) from bstart."""
                ms = {}
                avail = bstart
                for t in range(NBLK):
                    s = max(t + 2, avail)
                    ms.setdefault(s, []).append(t)
                    avail = s + 1 if len(ms[s]) >= 2 else s
                return ms

            def agg_sweep(l, back, back2, bstart, post_a=None, post_all=None):
                ms = mid_steps(bstart)
                mstep = {t: s for s, ts in ms.items() for t in ts}
                last_step = max(ms) + (2 if back2 else 0)
                for step in range(0, last_step + 1):
                    for t in ms.get(step, []):
                        agg_mid(l, t)
                    # back2 first: its inputs are 2 steps old, so it gives the
                    # DVE/PE queues work while back() waits on the B PSUM stop.
                    if back2 is not None:
                        for t in ms.get(step - 2, []):
                            back2(t)
                            if post_a is not None and t == NBLKA[1] - 1:
                                post_a()
                            if post_all is not None and t == NBLK - 1:
                                post_all()
                    for t in ms.get(step, []):
                        back(t)
                    if step < NBLK:
                        # direct accumulation once mids have caught up close
                        agg_front(
                            l,
                            step,
                            os.environ.get("KHYB", "1") == "1"
                            and mstep[step] - step <= 3,
                        )

            # ---- layer 0 build ----
            for t in range(NBLKA[0]):
                build0(t)
            allgather(0, 0)
            for t in range(NBLKA[0], NBLK):
                build0(t)
            allgather(0, 1)
            # ---- layer 0 aggregate + layer 1 build (fused) ----
            agg_sweep(
                0,
                agg0_back,
                agg0_back2,
                BST0,
                post_a=lambda: allgather(1, 0),
                post_all=lambda: allgather(1, 1),
            )
            # ---- layer 1 aggregate ----
            agg_sweep(1, agg1_back, None, BST1)

    nc.compile()
    return nc


_CACHE = {}


def kernel(**inputs):
    global LAST_RESULTS
    in_maps, nch, plain_ln = _host_prep(**inputs)
    key = (
        tuple(nch.reshape(-1).tolist()),
        plain_ln,
        os.environ.get("KB0"),
        os.environ.get("KB1"),
        os.environ.get("KSORT"),
        os.environ.get("KF8"),
        os.environ.get("KHYB"),
        os.environ.get("KTTR"),
    )
    if key not in _CACHE:
        _CACHE[key] = _build_program(nch, plain_ln)
    nc = _CACHE[key]
    trace = bool(os.environ.get("BASS_TRACE"))
    res = run_bass_kernel_spmd(nc, in_maps, list(range(NCORE)), trace=trace)
    LAST_RESULTS = res
    out = np.concatenate([res.results[c]["out"] for c in range(NCORE)], axis=0)
    return out.astype(np.float32)


# revision 17
# speedup vs baseline: 1.0465x; 1.0327x over previous
"""GAT-style 2-layer GNN message passing on 8 Trainium2 NeuronCores.

Math note: for this reference, the segment-softmax ratio
  num/den = (sum_j h[j]*exp((s_l[i]+s_r[j])/2d)) / (sum_j exp((s_l[i]+s_r[j])/2d))
has the destination factor exp(s_l[i]/2d) cancel, so per layer we only need
  a[i] = (sum_{j in N(i)} w_j*h_j) / (sum_{j in N(i)} w_j),  w_j = exp(s_r[j]/2d).

Sharding: nodes split into 8 contiguous destination ranges (6250/core).
Each core builds table rows [g=w*h (256) | w (4) | pad] (bf16, 768B) for its
own nodes; per layer the table is split in two halves by local row and
distributed with two pipelined AllGathers (layer 0 uses a small A half so
the first AllGather lands early; layer 1 uses a small B half so the last
AllGather before the final sweep is short). Each core aggregates its own
destinations: per-edge dma_gather of source rows (per-block chunk-exact
counts with -1 tail trim), then one-hot (fp8) matmul segment-sum into PSUM.
Early blocks (B-half gather still waiting on its AllGather) spill A-half
partials to an SBUF accumulator; steady-state blocks accumulate B directly
onto the A PSUM group. Layer-1 table rows are built inline in the layer-0
aggregation sweep via a PE transpose of x1.
"""

import os
import sys

import numpy as np
import ml_dtypes

sys.path.insert(0, "/opt/trn_rl_repo")

import concourse.bacc as bacc
import concourse.bass as bass
import concourse.mybir as mybir
import concourse.tile as tile
from concourse.bass_utils import run_bass_kernel_spmd

BF16 = mybir.dt.bfloat16
F32 = mybir.dt.float32
I16 = mybir.dt.int16
F8 = mybir.dt.float8e4 if os.environ.get("KF8", "1") == "1" else mybir.dt.bfloat16
F8R = mybir.dt.float8e4

N, DIN, E = 50000, 128, 800000
H, D = 4, 64
F = H * D  # 256
FH = F + H  # 260
NCORE = 8
NPC = N // NCORE  # 6250
NBLK = (NPC + 127) // 128  # 49 destination blocks per core
EPS = 1e-5
SLOPE = 0.01
ROWE = 512  # table row: 256 g + 4 (w-1) fp8 + 252 pad = 512 bytes
# per-layer (A, B) split of the 6250 local rows; 8*size must fit int16
SPLITS = ((2176, 4074), (3968, 2282))
K1 = (1 + SLOPE) / 2 / (2 * D)
K2 = (1 - SLOPE) / 2 / (2 * D)

LAST_RESULTS = None

AF = mybir.ActivationFunctionType
ALU = mybir.AluOpType


def _host_prep(x, edge_index, W0, b0, W1, b1, att0, att1, gamma, beta):
    """Build all per-core and shared input arrays.

    Per (layer, core, block, half) the edge list indexes into that layer's
    half table. All cores share one static chunk count per slot (the max
    over cores); cores short of (nch-1)*128+1 edges are topped up with
    row-0 dummy edges (one-hot zero), then -1-padded to nch*128 so the
    gather ucode trims the tail consistently with the ring reservation.
    """
    bf16 = ml_dtypes.bfloat16
    f8 = (
        ml_dtypes.float8_e4m3fn
        if os.environ.get("KF8", "1") == "1"
        else ml_dtypes.bfloat16
    )
    dst = np.asarray(edge_index[0], dtype=np.int64)
    src = np.asarray(edge_index[1], dtype=np.int64)

    plain_ln = bool(
        np.allclose(np.asarray(gamma), 1.0) and np.allclose(np.asarray(beta), 0.0)
    )

    ksort = os.environ.get("KSORT", "0") == "1"
    # per_core[l][c][b][half] -> (rows, lb)
    per_core = [[], []]
    for l in range(2):
        sa, sb = SPLITS[l]
        for c in range(NCORE):
            m = (dst >= c * NPC) & (dst < (c + 1) * NPC)
            ld = dst[m] - c * NPC
            s = src[m]
            owner = s // NPC
            srow = s - owner * NPC
            inA = srow < sa
            idxA_all = owner * sa + srow
            idxB_all = owner * sb + (srow - sa)
            blocks = []
            for b in range(NBLK):
                bm = (ld >> 7) == b
                lb = ld[bm] & 127
                a_m = inA[bm]
                halves = []
                for rows_all, hm in ((idxA_all[bm], a_m), (idxB_all[bm], ~a_m)):
                    rows = rows_all[hm]
                    ll = lb[hm]
                    if ksort:
                        order = np.argsort(rows, kind="stable")
                        rows, ll = rows[order], ll[order]
                    halves.append((rows, ll))
                blocks.append(halves)
            per_core[l].append(blocks)

    # static chunk counts (max over cores), per layer/block/half
    nch = np.zeros((2, NBLK, 2), dtype=np.int64)
    for l in range(2):
        for b in range(NBLK):
            for r in range(2):
                mx = max(len(per_core[l][c][b][r][0]) for c in range(NCORE))
                nch[l, b, r] = max(1, -(-mx // 128))
    eoff = [
        [np.concatenate([[0], np.cumsum(nch[l, :, r] * 8)]) for r in range(2)]
        for l in range(2)
    ]
    ohoff = [
        [np.concatenate([[0], np.cumsum(nch[l, :, r] * 128)]) for r in range(2)]
        for l in range(2)
    ]

    eidx_all = [[[], []], [[], []]]  # [l][r][c]
    ohm_all = [[[], []], [[], []]]
    for l in range(2):
        for c in range(NCORE):
            for r in range(2):
                eidx = np.zeros((128, int(eoff[l][r][-1])), dtype=np.int16)
                ohm = np.zeros((128, int(ohoff[l][r][-1])), dtype=f8)
                for b in range(NBLK):
                    rows, lb = per_core[l][c][b][r]
                    n = len(rows)
                    cap = int(nch[l, b, r]) * 128
                    floor = (int(nch[l, b, r]) - 1) * 128 + 1
                    ntop = max(n, min(floor, cap))
                    si = np.full(cap, -1, dtype=np.int16)
                    si[:n] = rows.astype(np.int16)
                    si[n:ntop] = 0
                    e0, e1 = int(eoff[l][r][b]), int(eoff[l][r][b + 1])
                    eidx[:, e0:e1] = np.tile(si.reshape(-1, 16).T, (8, 1))
                    col0 = int(ohoff[l][r][b])
                    pos = np.arange(n)
                    ohm[pos % 128, col0 + (pos // 128) * 128 + lb.astype(np.int64)] = 1.0
                eidx_all[l][r].append(eidx)
                ohm_all[l][r].append(ohm)

    cnt = np.zeros(N, np.float32)
    np.add.at(cnt, dst, 1.0)
    cnts = []
    for c in range(NCORE):
        pad = np.zeros(NBLK * 128, np.float32)
        pad[:NPC] = cnt[c * NPC : (c + 1) * NPC]
        cnts.append(np.ascontiguousarray(pad.reshape(NBLK, 128).T))
    xts = []
    xf = np.asarray(x, dtype=np.float32)
    for c in range(NCORE):
        xt = np.zeros((DIN, NBLK * 128), dtype=bf16)
        xt[:, :NPC] = xf[c * NPC : (c + 1) * NPC].T.astype(bf16)
        xts.append(xt)

    shared = {
        "w0t": np.ascontiguousarray(np.asarray(W0, np.float32).T).astype(bf16),
        "w1t": np.ascontiguousarray(np.asarray(W1, np.float32).T).astype(bf16),
        "b0b": np.tile(np.asarray(b0, np.float32)[None, :], (128, 1)),
        "b1b": np.tile(np.asarray(b1, np.float32)[None, :], (128, 1)),
        "ar0": np.tile(
            np.asarray(att0, np.float32)[0, :, D:].reshape(-1)[None, :], (128, 1)
        ),
        "ar1": np.tile(
            np.asarray(att1, np.float32)[0, :, D:].reshape(-1)[None, :], (128, 1)
        ),
        "gmb": np.tile(np.asarray(gamma, np.float32)[None, :], (128, 1)),
        "btb": np.tile(np.asarray(beta, np.float32)[None, :], (128, 1)),
        "idn": np.eye(128, dtype=np.float32).astype(bf16),
    }
    in_maps = []
    for c in range(NCORE):
        m = dict(shared)
        m["xt"] = xts[c]
        m["cnt"] = cnts[c]
        for l in range(2):
            for r, hn in ((0, "A"), (1, "B")):
                m[f"eidx{hn}{l}"] = eidx_all[l][r][c]
                m[f"ohm{hn}{l}"] = ohm_all[l][r][c]
        in_maps.append(m)
    return in_maps, nch, plain_ln


def _build_program(nch, plain_ln):
    """nch: [2, NBLK, 2] static chunk counts per (layer, block, half)."""
    NCHA_MAX = int(nch[:, :, 0].max())
    NCHB_MAX = int(nch[:, :, 1].max())
    e16off = [
        [np.concatenate([[0], np.cumsum(nch[l, :, r] * 8)]) for r in range(2)]
        for l in range(2)
    ]
    ohoff = [
        [np.concatenate([[0], np.cumsum(nch[l, :, r] * 128)]) for r in range(2)]
        for l in range(2)
    ]
    NBLKA = [SPLITS[l][0] // 128 for l in range(2)]
    BST0 = int(os.environ.get("KB0", "22"))  # layer-0 first B-gather step
    BST1 = int(os.environ.get("KB1", "10"))  # layer-1 first B-gather step

    nc = bacc.Bacc(
        "TRN2",
        target_bir_lowering=False,
        debug=False,
        num_devices=NCORE,
        num_swdge_queues=4,
    )

    xt_d = nc.dram_tensor("xt", [DIN, NBLK * 128], BF16, kind="ExternalInput")
    eidx_d = [[None, None], [None, None]]
    ohm_d = [[None, None], [None, None]]
    for l in range(2):
        for r, hn in ((0, "A"), (1, "B")):
            eidx_d[l][r] = nc.dram_tensor(
                f"eidx{hn}{l}", [128, int(e16off[l][r][-1])], I16, kind="ExternalInput"
            )
            ohm_d[l][r] = nc.dram_tensor(
                f"ohm{hn}{l}", [128, int(ohoff[l][r][-1])], F8, kind="ExternalInput"
            )
    w0t_d = nc.dram_tensor("w0t", [DIN, F], BF16, kind="ExternalInput")
    w1t_d = nc.dram_tensor("w1t", [F, F], BF16, kind="ExternalInput")
    b0b_d = nc.dram_tensor("b0b", [128, F], F32, kind="ExternalInput")
    b1b_d = nc.dram_tensor("b1b", [128, F], F32, kind="ExternalInput")
    ar0_d = nc.dram_tensor("ar0", [128, F], F32, kind="ExternalInput")
    ar1_d = nc.dram_tensor("ar1", [128, F], F32, kind="ExternalInput")
    gmb_d = nc.dram_tensor("gmb", [128, F], F32, kind="ExternalInput")
    btb_d = nc.dram_tensor("btb", [128, F], F32, kind="ExternalInput")
    idn_d = nc.dram_tensor("idn", [128, 128], BF16, kind="ExternalInput")
    cnt_d = nc.dram_tensor("cnt", [128, NBLK], F32, kind="ExternalInput")
    out_d = nc.dram_tensor("out", [NPC, D], F32, kind="ExternalOutput")

    tbl_own = [
        [
            nc.dram_tensor(f"tbl_own{l}{r}", [SPLITS[l][r], ROWE], F8R)
            for r in range(2)
        ]
        for l in range(2)
    ]
    tbl_full = [
        [
            nc.dram_tensor(
                f"tbl_full{l}{r}", [NCORE * SPLITS[l][r], ROWE], F8R,
                addr_space="Shared",
            )
            for r in range(2)
        ]
        for l in range(2)
    ]

    groups = [list(range(NCORE))]

    with tile.TileContext(nc) as tc:
        with (
            tc.tile_pool(name="const", bufs=1) as cpool,
            tc.tile_pool(name="hbuf", bufs=3) as hpool,
            tc.tile_pool(name="small", bufs=6) as smpool,
            tc.tile_pool(name="tblt", bufs=3) as tbpool,
            tc.tile_pool(name="ohp", bufs=3) as ohpool,
            tc.tile_pool(name="post", bufs=3) as postpool,
            tc.tile_pool(name="xtp", bufs=4) as xtpool,
            tc.tile_pool(name="gemm", bufs=1, space="PSUM") as gpsum,
            tc.tile_pool(name="agga", bufs=3, space="PSUM") as apsumA,
            tc.tile_pool(name="aggb", bufs=2, space="PSUM") as apsumB,
            tc.tile_pool(name="tp", bufs=2, space="PSUM") as tpsum,
        ):
            # ---- load constants ----
            def cload(dram, shape, dtype):
                t = cpool.tile(shape, dtype, tag=dram.name)
                nc.sync.dma_start(out=t[:], in_=dram[:, :])
                return t

            xt_s = cload(xt_d, [DIN, NBLK * 128], BF16)
            eidx_s = [
                [
                    cload(eidx_d[l][r], [128, int(e16off[l][r][-1])], I16)
                    for r in range(2)
                ]
                for l in range(2)
            ]
            w0t_s = cload(w0t_d, [DIN, F], BF16)
            w1t_s = []
            for cch in range(2):
                t = cpool.tile([128, F], BF16, tag=f"w1t{cch}")
                nc.sync.dma_start(out=t[:], in_=w1t_d[cch * 128 : (cch + 1) * 128, :])
                w1t_s.append(t)
            b0b_s = cload(b0b_d, [128, F], F32)
            b1b_s = cload(b1b_d, [128, F], F32)
            ar0_s = cload(ar0_d, [128, F], F32)
            ar1_s = cload(ar1_d, [128, F], F32)
            if not plain_ln:
                gmb_s = cload(gmb_d, [128, F], F32)
                btb_s = cload(btb_d, [128, F], F32)
            idn_s = cload(idn_d, [128, 128], BF16)
            cnt_s = cload(cnt_d, [128, NBLK], F32)
            epsb_s = cpool.tile([128, 1], F32, tag="epsb")
            nc.vector.memset(epsb_s[:], EPS)
            # A-half spill accumulator, one [128, FH] f32 slab per block
            acc_s = cpool.tile([128, NBLK, FH], F32, tag="acc")

            gtA = []
            gtB = []
            for i in range(3):
                t = cpool.tile([128, NCHA_MAX, ROWE], F8R, tag=f"gtA{i}")
                nc.vector.memset(t[:], 0.0)
                gtA.append(t)
                t = cpool.tile([128, NCHB_MAX, ROWE], F8R, tag=f"gtB{i}")
                nc.vector.memset(t[:], 0.0)
                gtB.append(t)

            regs = {}
            vals = set()
            for v in nch.reshape(-1).tolist():
                v = int(v)
                vals.add(v)
                if v >= 2:
                    vals.add(v // 2)
                    vals.add(v - v // 2)
            for v in sorted(vals):
                regs[v] = nc.gpsimd.to_reg(v * 128)

            def att_scores(u, dst_ap):
                """dst = su + (K2/K1)*sa, where su/sa are +/- abs row sums of u."""
                su = smpool.tile([128, H], F32, tag="su")
                nc.vector.tensor_reduce(
                    su[:],
                    u[:].rearrange("p (h d) -> p h d", d=D),
                    axis=mybir.AxisListType.X,
                    op=ALU.add,
                )
                sa = smpool.tile([128, H], F32, tag="sa")
                nc.vector.tensor_reduce(
                    sa[:],
                    u[:].rearrange("p (h d) -> p h d", d=D),
                    axis=mybir.AxisListType.X,
                    op=ALU.add,
                    apply_absolute_value=True,
                )
                t1 = smpool.tile([128, H], F32, tag="t1")
                nc.vector.tensor_scalar(t1[:], sa[:], K2 / K1, None, op0=ALU.mult)
                nc.vector.tensor_tensor(dst_ap, su[:], t1[:], op=ALU.add)

            def emit_rows(l, t, h, srf):
                """Build [g=w*h | w] row block and DMA it to the own table."""
                rows = min(128, NPC - t * 128)
                sa = SPLITS[l][0]
                tb = tbpool.tile([128, F + H], F8R, tag="tb")
                wv = smpool.tile([128, H], F32, tag="wv")
                nc.scalar.activation(wv[:], srf, AF.Exp, scale=K1)
                nc.vector.tensor_scalar(
                    tb[:, F : F + H], wv[:], -1.0, None, op0=ALU.add
                )
                nc.vector.tensor_tensor(
                    tb[:, 0:F].rearrange("p (h d) -> p h d", d=D),
                    h[:].rearrange("p (h d) -> p h d", d=D),
                    wv[:].to_broadcast((128, H, D)),
                    op=ALU.mult,
                )
                if t < NBLKA[l]:
                    dst = tbl_own[l][0][t * 128 : t * 128 + rows, 0 : F + H]
                else:
                    r0 = t * 128 - sa
                    dst = tbl_own[l][1][r0 : r0 + rows, 0 : F + H]
                nc.sync.dma_start(out=dst, in_=tb[:rows, :])

            def build0(t):
                """Layer-0 GEMM + table row for destination block t."""
                ps = gpsum.tile([128, F], F32, tag="gemm")
                nc.tensor.matmul(
                    ps[:],
                    lhsT=xt_s[:, t * 128 : (t + 1) * 128],
                    rhs=w0t_s[:],
                    start=True,
                    stop=True,
                )
                h = hpool.tile([128, F], F32, tag="h")
                nc.vector.tensor_tensor(h[:], ps[:], b0b_s[:], op=ALU.add)
                u = hpool.tile([128, F], F32, tag="u")
                nc.vector.tensor_tensor(u[:], h[:], ar0_s[:], op=ALU.mult)
                srf = smpool.tile([128, H], F32, tag="srf")
                att_scores(u, srf[:])
                emit_rows(0, t, h, srf[:])

            qctr = [0]

            def gather_half(l, t, r):
                """Issue the (r 0 = half A, 1 = half B) gather for block t,
                split into two half-size gathers on different SWDGE queues so
                all four queue core-pairs stay busy."""
                gt = (gtA if r == 0 else gtB)[t % 3]
                nchv = int(nch[l, t, r])
                e0 = int(e16off[l][r][t])
                if os.environ.get("KSPL", "1") == "1" and nchv >= 2:
                    pieces = [(0, nchv // 2), (nchv // 2, nchv)]
                else:
                    pieces = [(0, nchv)]
                for c0, c1 in pieces:
                    qn = qctr[0] % 4
                    qctr[0] += 1
                    w = c1 - c0
                    nc.gpsimd.dma_gather(
                        gt[:, c0:c1, :],
                        tbl_full[l][r][:, :],
                        eidx_s[l][r][:, e0 + c0 * 8 : e0 + c1 * 8],
                        w * 128,
                        regs[w],
                        ROWE,
                        single_packet=(w * 128 <= 1024),
                        queue_num=qn,
                    )
                return gt

            def load_onehot(l, t, r):
                mx = NCHA_MAX if r == 0 else NCHB_MAX
                nchv = int(nch[l, t, r])
                oh = ohpool.tile([128, mx * 128], F8, tag=f"oh{r}")
                o0 = int(ohoff[l][r][t])
                nc.sync.dma_start(
                    out=oh[:, 0 : nchv * 128], in_=ohm_d[l][r][:, o0 : o0 + nchv * 128]
                )
                return oh

            def agg_matmuls(ps, gt, oh, l, t, r, start, stop):
                nchv = int(nch[l, t, r])
                for b in range(nchv):
                    nc.tensor.matmul(
                        ps[:],
                        lhsT=oh[:, b * 128 : (b + 1) * 128],
                        rhs=gt[:, b, 0:FH],
                        start=(start and b == 0),
                        stop=(stop and b == nchv - 1),
                    )

            ps_live = {}
            direct = {}

            def agg_front(l, t, is_direct):
                """A-half gather + matmuls; spill to SBUF unless direct."""
                ga = gather_half(l, t, 0)
                oh = load_onehot(l, t, 0)
                ps = apsumA.tile([128, FH], F32, tag="aggA")
                agg_matmuls(ps, ga, oh, l, t, 0, start=True, stop=not is_direct)
                direct[t] = is_direct
                if is_direct:
                    ps_live[t] = ps
                else:
                    nc.vector.tensor_copy(acc_s[:, t, :], ps[:])

            def agg_mid(l, t):
                gb = gather_half(l, t, 1)
                oh = load_onehot(l, t, 1)
                if direct[t]:
                    agg_matmuls(ps_live[t], gb, oh, l, t, 1, start=False, stop=True)
                else:
                    ps = apsumB.tile([128, FH], F32, tag="aggB")
                    agg_matmuls(ps, gb, oh, l, t, 1, start=True, stop=True)
                    ps_live[t] = ps

            def div_merge(t):
                """-> (num/den source tile, rec).  Merges the spill if needed."""
                ps = ps_live.pop(t)
                if direct.pop(t):
                    src = ps
                else:
                    s = postpool.tile([128, FH], F32, tag="s")
                    nc.vector.tensor_tensor(s[:], ps[:], acc_s[:, t, :], op=ALU.add)
                    src = s
                dn = smpool.tile([128, H], F32, tag="dn")
                nc.vector.tensor_tensor(
                    dn[:], src[:, F:FH], cnt_s[:, t : t + 1].to_broadcast((128, H)),
                    op=ALU.add,
                )
                rec = smpool.tile([128, H], F32, tag="rec")
                nc.vector.reciprocal_approx_fast(out=rec[:], in_=dn[:])
                return src, rec

            x1T_live = {}

            def agg0_back(t):
                """num/den + LayerNorm + lrelu; PE-transpose x1 for the GEMM."""
                s, rec = div_merge(t)
                a0 = postpool.tile([128, F], F32, tag="a0")
                sm = smpool.tile([128, 1], F32, tag="sm")
                scr = postpool.tile([128, F], F32, tag="scr")
                sq = smpool.tile([128, 1], F32, tag="sq")
                if os.environ.get("KTTR", "0") == "1":
                    nc.vector.tensor_tensor_reduce(
                        a0[:].rearrange("p (h d) -> p h d", d=D),
                        s[:, 0:F].rearrange("p (h d) -> p h d", d=D),
                        rec[:].to_broadcast((128, H, D)),
                        1.0,
                        0.0,
                        op0=ALU.mult,
                        op1=ALU.add,
                        accum_out=sm[:],
                    )
                    nc.vector.tensor_tensor_reduce(
                        scr[:], a0[:], a0[:], 1.0, 0.0,
                        op0=ALU.mult, op1=ALU.add, accum_out=sq[:],
                    )
                else:
                    nc.vector.tensor_tensor(
                        a0[:].rearrange("p (h d) -> p h d", d=D),
                        s[:, 0:F].rearrange("p (h d) -> p h d", d=D),
                        rec[:].to_broadcast((128, H, D)),
                        op=ALU.mult,
                    )
                    nc.vector.tensor_reduce(
                        sm[:], a0[:], axis=mybir.AxisListType.X, op=ALU.add
                    )
                    nc.vector.tensor_tensor(scr[:], a0[:], a0[:], op=ALU.mult)
                    nc.vector.tensor_reduce(
                        sq[:], scr[:], axis=mybir.AxisListType.X, op=ALU.add
                    )
                mun = smpool.tile([128, 1], F32, tag="mun")
                nc.vector.tensor_scalar(mun[:], sm[:], -1.0 / F, None, op0=ALU.mult)
                m2 = smpool.tile([128, 1], F32, tag="m2")
                nc.vector.tensor_tensor(m2[:], mun[:], sm[:], op=ALU.mult)
                dv = smpool.tile([128, 1], F32, tag="dv")
                nc.vector.tensor_tensor(dv[:], sq[:], m2[:], op=ALU.add)
                rstd = smpool.tile([128, 1], F32, tag="rstd")
                nc.scalar.activation(
                    rstd[:], dv[:], AF.Abs_reciprocal_sqrt, bias=epsb_s[:], scale=1.0 / F
                )
                nmr = smpool.tile([128, 1], F32, tag="nmr")
                nc.vector.tensor_tensor(nmr[:], mun[:], rstd[:], op=ALU.mult)
                x1b = tbpool.tile([128, F], BF16, tag="x1b")
                if plain_ln:
                    nc.scalar.activation(
                        x1b[:], a0[:], AF.Lrelu, bias=nmr[:], scale=rstd[:], alpha=SLOPE
                    )
                else:
                    xn = postpool.tile([128, F], F32, tag="xn")
                    nc.scalar.activation(xn[:], a0[:], AF.Copy, bias=nmr[:], scale=rstd[:])
                    xg = postpool.tile([128, F], F32, tag="xg")
                    nc.vector.tensor_tensor(xg[:], xn[:], gmb_s[:], op=ALU.mult)
                    xgb = postpool.tile([128, F], F32, tag="xgb")
                    nc.vector.tensor_tensor(xgb[:], xg[:], btb_s[:], op=ALU.add)
                    nc.scalar.activation(x1b[:], xgb[:], AF.Lrelu, alpha=SLOPE)
                pt = tpsum.tile([128, F], BF16, tag="tp")
                for cch in range(2):
                    nc.tensor.transpose(
                        pt[:, cch * 128 : (cch + 1) * 128],
                        x1b[:, cch * 128 : (cch + 1) * 128],
                        idn_s[:],
                    )
                xtT = xtpool.tile([128, F], BF16, tag="xT")
                nc.vector.tensor_copy(xtT[:], pt[:])
                x1T_live[t] = xtT

            def agg0_back2(t):
                """Layer-1 GEMM from the transposed x1 + scores + table rows."""
                xtT = x1T_live.pop(t)
                ps2 = gpsum.tile([128, F], F32, tag="gemm")
                for cch in range(2):
                    nc.tensor.matmul(
                        ps2[:],
                        lhsT=xtT[:, cch * 128 : (cch + 1) * 128],
                        rhs=w1t_s[cch][:],
                        start=(cch == 0),
                        stop=(cch == 1),
                    )
                h1 = hpool.tile([128, F], F32, tag="h")
                nc.vector.tensor_tensor(h1[:], ps2[:], b1b_s[:], op=ALU.add)
                u1 = hpool.tile([128, F], F32, tag="u")
                nc.vector.tensor_tensor(u1[:], h1[:], ar1_s[:], op=ALU.mult)
                srf = smpool.tile([128, H], F32, tag="srf")
                att_scores(u1, srf[:])
                emit_rows(1, t, h1, srf[:])

            def agg1_back(t):
                """num/den then head mean for block t."""
                s, rec = div_merge(t)
                rows = min(128, NPC - t * 128)
                rec4 = smpool.tile([128, H], F32, tag="rec4")
                nc.vector.tensor_scalar(rec4[:], rec[:], 0.25, None, op0=ALU.mult)
                q = postpool.tile([128, F], F32, tag="a0")
                nc.vector.tensor_tensor(
                    q[:].rearrange("p (h d) -> p h d", d=D),
                    s[:, 0:F].rearrange("p (h d) -> p h d", d=D),
                    rec4[:].to_broadcast((128, H, D)),
                    op=ALU.mult,
                )
                p01 = postpool.tile([128, D], F32, tag="p01")
                nc.vector.tensor_tensor(p01[:], q[:, 0:D], q[:, D : 2 * D], op=ALU.add)
                p23 = postpool.tile([128, D], F32, tag="p23")
                nc.vector.tensor_tensor(
                    p23[:], q[:, 2 * D : 3 * D], q[:, 3 * D : 4 * D], op=ALU.add
                )
                o = postpool.tile([128, D], F32, tag="o")
                nc.vector.tensor_tensor(o[:], p01[:], p23[:], op=ALU.add)
                nc.sync.dma_start(
                    out=out_d[t * 128 : t * 128 + rows, :], in_=o[:rows, :]
                )

            def allgather(l, half):
                nc.gpsimd.collective_compute(
                    "AllGather",
                    ALU.bypass,
                    replica_groups=groups,
                    ins=[tbl_own[l][half][:, :]],
                    outs=[tbl_full[l][half][:, :]],
                )

            # ================= schedule =================
            def mid_steps(bstart):
                """mid step per block: catch-up (2 mids/step) from bstart."""
                ms = {}
                avail = bstart
                for t in range(NBLK):
                    s = max(t + 2, avail)
                    ms.setdefault(s, []).append(t)
                    avail = s + 1 if len(ms[s]) >= 2 else s
                return ms

            def agg_sweep(l, back, back2, bstart, post_a=None, post_all=None):
                ms = mid_steps(bstart)
                mstep = {t: s for s, ts in ms.items() for t in ts}
                last_step = max(ms) + (2 if back2 else 0)
                for step in range(0, last_step + 1):
                    for t in ms.get(step, []):
                        agg_mid(l, t)
                    # back2 first: its inputs are 2 steps old, so it gives the
                    # DVE/PE queues work while back() waits on the B PSUM stop.
                    if back2 is not None:
                        for t in ms.get(step - 2, []):
                            back2(t)
                            if post_a is not None and t == NBLKA[1] - 1:
                                post_a()
                            if post_all is not None and t == NBLK - 1:
                                post_all()
                    for t in ms.get(step, []):
                        back(t)
                    if step < NBLK:
                        # direct accumulation once mids have caught up close
                        agg_front(
                            l,
                            step,
                            os.environ.get("KHYB", "1") == "1"
                            and mstep[step] - step <= 3,
                        )

            # ---- layer 0 build ----
            for t in range(NBLKA[0]):
                build0(t)
            allgather(0, 0)
            for t in range(NBLKA[0], NBLK):
                build0(t)
            allgather(0, 1)
            # ---- layer 0 aggregate + layer 1 build (fused) ----
            agg_sweep(
                0,
                agg0_back,
                agg0_back2,
                BST0,
                post_a=lambda: allgather(1, 0),
                post_all=lambda: allgather(1, 1),
            )
            # ---- layer 1 aggregate ----
            agg_sweep(1, agg1_back, None, BST1)

    nc.compile()
    return nc


_CACHE = {}


def kernel(**inputs):
    global LAST_RESULTS
    in_maps, nch, plain_ln = _host_prep(**inputs)
    key = (
        tuple(nch.reshape(-1).tolist()),
        plain_ln,
        os.environ.get("KB0"),
        os.environ.get("KB1"),
        os.environ.get("KSORT"),
        os.environ.get("KF8"),
        os.environ.get("KHYB"),
        os.environ.get("KTTR"),
        os.environ.get("KSPL"),
    )
    if key not in _CACHE:
        _CACHE[key] = _build_program(nch, plain_ln)
    nc = _CACHE[key]
    trace = bool(os.environ.get("BASS_TRACE"))
    res = run_bass_kernel_spmd(nc, in_maps, list(range(NCORE)), trace=trace)
    LAST_RESULTS = res
    out = np.concatenate([res.results[c]["out"] for c in range(NCORE)], axis=0)
    return out.astype(np.float32)
